# revision 40
# baseline (speedup 1.0000x reference)
"""Trainium2 Bass kernel for nn_AttentionOperation (sparse_attention).

Computation (per the reference):
    sim  = QK^T                  [N,H,L,L]
    sim  = BN_heads(sim)         (stats over b,l,m per head)
    attn = softmax(sim, -1)
    rv   = attn @ V^T            [N,H,C,L] -> [N, H*C, L]
    rv   = BN_channels(rv)       (stats over b,l per channel)
    out  = gelu_exact(rv)

Sharding: one head per NeuronCore (H=8, n_cores=8).  Both BatchNorms are
then fully core-local (sim-BN stats are per head; val-BN channels
h*64..(h+1)*64-1 belong exactly to head h), so there is no communication.

Device-side structure (PE runs mostly at the throttled 1.2 GHz clock, so
the pair-stacked QK layout matters: batch b_in=0 lives on partitions
0:64 and b_in=1 on 64:128, which the PE runs as concurrent row-tiles):
  * BN1 mean/bias shift cancels inside the softmax, so only
    g = w_h * rsqrt(var + eps) is needed.  var comes from tiny Gram
    matmuls; kqo arrives in two per-batch-pair DMAs so the grams overlap
    the transfer of the second half.
  * matmul operands are fp16 (fp32 matmul = 2 half-rate passes on PE).
  * softmax denominator comes free from a ones-row appended to V^T.
  * rsqrt is a DVE-only quake-seed Newton iteration (2 its for BN1, 1
    for BN2 - seed err 3.4%, 1 it -> 1.7e-3), so ScalarE needs only the
    Exp and Gelu table sets (2 loads total).
  * BN2 affine is folded into the Gelu activation's scale/bias operands.
  * output is fp16 (host upcasts) and leaves via per-batch sync-ring
    DMAs that pipeline under the remaining gelus.
"""

import numpy as np

N, H, D, L = 4, 8, 64, 1024
C = 64
NCH = L // 128  # m-chunks of 128
EPS = 1e-3
NG = 8   # m-chunks in the BN1 gram stats (8 = exact reference stats)
CNT = float(N * (128 * NG) * (128 * NG))  # sampled sim pairs

_CACHE = {}


def _build_nc():
    import concourse.bacc as bacc
    import concourse.tile as tile
    import concourse.mybir as mybir

    f32 = mybir.dt.float32
    f16 = mybir.dt.float16
    i32 = mybir.dt.int32
    AF = mybir.ActivationFunctionType
    ALU = mybir.AluOpType

    nc = bacc.Bacc("TRN2", target_bir_lowering=False, debug=False)

    q2_d = nc.dram_tensor("q2", [128, 2, L], f16, kind="ExternalInput")
    k2_d = nc.dram_tensor("k2", [128, 2, L], f16, kind="ExternalInput")
    kqo_d = nc.dram_tensor("kqo", [128, N, NG, 129], f16,
                           kind="ExternalInput")
    vo_d = nc.dram_tensor("vo", [128, N, NCH, 65], f16, kind="ExternalInput")
    id_d = nc.dram_tensor("ident", [128, 64], f16, kind="ExternalInput")
    ws_d = nc.dram_tensor("ws", [1, 1], f32, kind="ExternalInput")
    wv_d = nc.dram_tensor("wv", [64, 1], f32, kind="ExternalInput")
    bv_d = nc.dram_tensor("bv", [64, 1], f32, kind="ExternalInput")
    out_d = nc.dram_tensor("out", [64, N, L], f16, kind="ExternalOutput")

    with tile.TileContext(nc) as tc:
        with (
            tc.tile_pool(name="cst", bufs=1) as cst,
            tc.tile_pool(name="sm", bufs=1) as sm,
            tc.tile_pool(name="exp", bufs=24) as epool,
            tc.tile_pool(name="rvp", bufs=4) as rvp,
            tc.tile_pool(name="outp", bufs=4) as outp,
            tc.tile_pool(name="ps", bufs=1, space="PSUM") as psp,
        ):
            ones64 = cst.tile([64, 1], f32)
            nc.vector.memset(ones64[:], 1.0)
            magic1 = cst.tile([1, 1], i32)
            nc.vector.memset(magic1[:], 0x5F3759DF)
            magic64 = cst.tile([64, 1], i32)
            nc.vector.memset(magic64[:], 0x5F3759DF)
            # dummy exp so the ACT exp-table load happens off the critical
            # path (otherwise it lands right before the first real exp)
            warm_sb = sm.tile([1, 1], f32, tag="warm", bufs=1)
            nc.scalar.activation(warm_sb[:], ones64[0:1, 0:1], AF.Exp)

            # ---- input DMAs.  The sync ring drains FIFO, so issue in
            # dependency order: kqo halves (gate the grams), then pair-0
            # q/k (gates the hoisted first QK chunks), then the tiny
            # tensors the g-chain needs, then the rest.
            kqo_sb = cst.tile([128, N, NG, 129], f16)
            nc.sync.dma_start(kqo_sb[:, 0:2], kqo_d.ap()[:, 0:2])
            # second half on the scalar HWDGE ring: the halves transfer
            # concurrently, so grams for batches 2/3 start ~1.5us earlier
            nc.scalar.dma_start(kqo_sb[:, 2:4], kqo_d.ap()[:, 2:4])
            id_sb = cst.tile([128, 64], f16)
            nc.sync.dma_start(id_sb[:], id_d.ap())
            ws_sb = cst.tile([1, 1], f32)
            nc.sync.dma_start(ws_sb[:], ws_d.ap())
            q2_sb = cst.tile([128, 2, L], f16)
            k2_sb = cst.tile([128, 2, L], f16)
            nc.sync.dma_start(k2_sb[:, 0], k2_d.ap()[:, 0])
            nc.sync.dma_start(q2_sb[:, 0], q2_d.ap()[:, 0])
            nc.sync.dma_start(k2_sb[:, 1], k2_d.ap()[:, 1])
            nc.sync.dma_start(q2_sb[:, 1], q2_d.ap()[:, 1])
            wv_sb = cst.tile([64, 1], f32)
            nc.sync.dma_start(wv_sb[:], wv_d.ap())
            bv_sb = cst.tile([64, 1], f32)
            nc.sync.dma_start(bv_sb[:], bv_d.ap())
            vo_sb = cst.tile([128, N, NCH, 65], f16)
            nc.sync.dma_start(vo_sb[:, 0:2], vo_d.ap()[:, 0:2])
            nc.sync.dma_start(vo_sb[:, 2:4], vo_d.ap()[:, 2:4])

            # ---- BN1 stats: one stacked gram matmul per (batch, chunk).
            # G[b] = [k|q|1]^T [k|q|1]:  C_K = G[0:64,0:64],
            # C_Q = G[64:128,64:128], ksum = G[0:64,128], qsum = G[64:128,128]
            # Everything is done per batch so only batch 3's realign +
            # product chain sits on the critical path after its gram.
            qsp = sm.tile([64, 2], f32, tag="qs", bufs=1)
            sprod = sm.tile([64, N], f32, tag="sprod", bufs=1)
            kk_sb = sm.tile([64, N, 129], f32, tag="kk", bufs=1)
            qsrc_sb = sm.tile([128, N, 65], f16, tag="gk", bufs=1)
            for b in range(N):
                gps = psp.tile([128, 129], f32, tag="av", bufs=2,
                               name=f"gram_ps_{b}")
                for c in range(NG):
                    nc.tensor.matmul(
                        gps[:], kqo_sb[:, b, c, 0:128], kqo_sb[:, b, c, :],
                        start=(c == 0), stop=(c == NG - 1))
                nc.scalar.copy(kk_sb[:, b, :], gps[0:64, :])
                nc.vector.tensor_copy(qsrc_sb[64:128, b, :],
                                      gps[64:128, 64:129])
            qq_ps = psp.tile([64, N, 65], f32, tag="av", bufs=2, name="qq")
            nc.tensor.matmul(qq_ps[:], id_sb[64:128, :],
                             qsrc_sb[64:128, :, :], start=True, stop=True)
            # products read C_Q straight from PSUM (DVE has a PSUM port)
            pscr = sm.tile([64, N, 64], f32, tag="pscr", bufs=1)
            nc.vector.tensor_tensor(
                out=pscr[:], in0=kk_sb[:, :, 0:64], in1=qq_ps[:, :, 0:64],
                op=ALU.mult)
            nc.vector.tensor_reduce(
                out=qsp[:, 0:1], in_=pscr[:],
                axis=mybir.AxisListType.XY, op=ALU.add)
            nc.vector.tensor_tensor(
                out=sprod[:], in0=kk_sb[:, :, 128], in1=qq_ps[:, :, 64],
                op=ALU.mult)
            nc.vector.tensor_reduce(
                out=qsp[:, 1:2], in_=sprod[:],
                axis=mybir.AxisListType.X, op=ALU.add)

            # hoisted first QK chunks: keeps PE busy while the g-chain
            # (DVE scalar ops) runs, and has sim ready for the first exps
            def emit_qk(pair, c):
                sims = []
                for b_in in range(2):
                    b = 2 * pair + b_in
                    r0 = 64 * b_in
                    sim_ps = psp.tile([128, L], f32, tag="sim", bufs=3,
                                      name=f"sim_ps_{b}_{c}")
                    for half in range(2):
                        nc.tensor.matmul(
                            sim_ps[:, 512 * half:512 * (half + 1)],
                            k2_sb[r0:r0 + 64, pair, 128 * c:128 * (c + 1)],
                            q2_sb[r0:r0 + 64, pair,
                                  512 * half:512 * (half + 1)],
                            start=True, stop=True)
                    sims.append(sim_ps)
                return sims

            pre_sims = [emit_qk(0, 0)]

            # partition-sum via PE, then fold the 4 per-batch slots:
            # qs2 [1,2] = [sum(sim^2), sum(sim)]
            scps = psp.tile([1, 2], f32, tag="av", bufs=2)
            nc.tensor.matmul(scps[:], ones64[:], qsp[:], start=True,
                             stop=True)
            qs2 = sm.tile([1, 2], f32, tag="qs2", bufs=1)
            nc.vector.tensor_copy(qs2[:], scps[:])

            # DVE-only rsqrt(x + eps): quake seed + Newton iterations.
            def dve_rsqrt(dst_ap, x_ap, p, magic, n_it, pref,
                          add_eps=True):
                if add_eps:
                    xe_t = sm.tile([p, 1], f32, tag=f"{pref}xe", bufs=1,
                                   name=f"{pref}_xe")
                    nc.vector.tensor_scalar_add(xe_t[:], x_ap, EPS)
                    xe = xe_t[:]
                else:
                    xe = x_ap
                sh = sm.tile([p, 1], i32, tag=f"{pref}sh", bufs=1,
                             name=f"{pref}_sh")
                nc.vector.tensor_scalar(
                    out=sh[:], in0=xe.bitcast(i32), scalar1=1,
                    scalar2=None, op0=ALU.arith_shift_right)
                y = sm.tile([p, 1], f32, tag=f"{pref}y", bufs=1,
                            name=f"{pref}_y")
                nc.vector.tensor_tensor(out=y[:].bitcast(i32), in0=magic[:],
                                        in1=sh[:], op=ALU.subtract)
                t = sm.tile([p, 1], f32, tag=f"{pref}t", bufs=1,
                            name=f"{pref}_t")
                for it in range(n_it):
                    nc.vector.tensor_tensor(out=t[:], in0=y[:], in1=y[:],
                                            op=ALU.mult)
                    nc.vector.scalar_tensor_tensor(
                        out=t[:], in0=t[:], scalar=-0.5, in1=xe,
                        op0=ALU.mult, op1=ALU.mult)
                    nc.vector.scalar_tensor_tensor(
                        out=(dst_ap if it == n_it - 1 else y[:]), in0=t[:],
                        scalar=1.5, in1=y[:], op0=ALU.add, op1=ALU.mult)

            # var = E[x^2] - E[x]^2 ; g = w_h * rsqrt(var + eps)
            m2_t = sm.tile([1, 1], f32, tag="sc1", bufs=1)
            nc.vector.scalar_tensor_tensor(
                out=m2_t[:], in0=qs2[:, 1:2], scalar=1.0 / (CNT * CNT),
                in1=qs2[:, 1:2], op0=ALU.mult, op1=ALU.mult)
            var_t = sm.tile([1, 1], f32, tag="sc2", bufs=1)
            nc.vector.scalar_tensor_tensor(
                out=var_t[:], in0=qs2[:, 0:1], scalar=1.0 / CNT,
                in1=m2_t[:], op0=ALU.mult, op1=ALU.subtract)
            rs_t = sm.tile([1, 1], f32, tag="sc3", bufs=1)
            dve_rsqrt(rs_t[:], var_t[:], 1, magic1, 2, "g",
                      add_eps=False)
            g_t = sm.tile([1, 1], f32, tag="sc4", bufs=1)
            nc.vector.tensor_tensor(out=g_t[:], in0=rs_t[:], in1=ws_sb[:],
                                    op=ALU.mult)
            # broadcast g to all 128 partitions (gpsimd, ~300ns)
            g128 = cst.tile([128, 1], f32)
            nc.gpsimd.partition_broadcast(g128[:], g_t[:], channels=128)

            # ---- main attention pipeline ----
            exp_tiles = [[None] * NCH for _ in range(N)]
            rv_tiles = []
            stats = cst.tile([64, 2 * N, 6], f32)

            def emit_exp(pair, c, sims):
                for b_in in range(2):
                    b = 2 * pair + b_in
                    ex = epool.tile([128, L], f16, tag="exp", bufs=24,
                                    name=f"exp_{b}_{c}")
                    nc.scalar.activation(ex[:], sims[b_in][:], AF.Exp,
                                         scale=g128[:, 0:1])
                    exp_tiles[b][c] = ex

            def emit_avs(b, rv_pair):
                    b_in = b % 2
                    rcp_sb = sm.tile([1, L], f32, tag="rcp", bufs=4,
                                     name=f"rcp_{b}")
                    rbc_sb = sm.tile([64, L], f32, tag="rbc", bufs=4,
                                     name=f"rbc_{b}")
                    rv_sb = rv_pair[:, b_in]
                    for half in range(2):
                        hs = slice(512 * half, 512 * (half + 1))
                        av_ps = psp.tile([65, 512], f32, tag="av", bufs=2,
                                         name=f"av_ps_{b}_{half}")
                        for c in range(NCH):
                            nc.tensor.matmul(
                                av_ps[:], vo_sb[:, b, c, :],
                                exp_tiles[b][c][:, hs],
                                start=(c == 0), stop=(c == NCH - 1))
                        # den row to partition 0 (custom-DVE recip needs a
                        # base-0 SBUF operand), then a full-tile copy --
                        # same DVE cost as a 1-row copy (free-dim-bound) --
                        # to release the PSUM slot for the next batch's AV
                        den_sb = sm.tile([1, 512], f32, tag="den", bufs=4,
                                         name=f"den_{b}_{half}")
                        nc.vector.tensor_copy(den_sb[:], av_ps[64:65, :])
                        if b < N - 1:
                            # early PSUM release matters only when another
                            # batch's AV still needs the slot
                            av_sb = sm.tile([65, 512], f32, tag="avs",
                                            bufs=6, name=f"av_sb_{b}_{half}")
                            nc.vector.tensor_copy(av_sb[:], av_ps[:])
                            num = av_sb[0:64, :]
                        else:
                            num = av_ps[0:64, :]
                        nc.vector.reciprocal_approx_fast(
                            out=rcp_sb[0:1, hs], in_=den_sb[:])
                        nc.gpsimd.partition_broadcast(
                            rbc_sb[:, hs], rcp_sb[0:1, hs], channels=64)
                        nc.vector.tensor_tensor(
                            out=rv_sb[:, hs], in0=num,
                            in1=rbc_sb[:, hs], op=ALU.mult)
                        nc.vector.bn_stats(stats[:, 2 * b + half, :],
                                           rv_sb[:, hs])

            done = set()
            rv_pairs = [rvp.tile([64, 2, L], f32, tag="rv", bufs=2,
                                 name=f"rv_pair_{p}") for p in range(2)]
            for pair in range(2):
                rv_tiles.append(rv_pairs[pair])
                for c in range(NCH):
                    if (pair, c) in done:
                        continue
                    if pair == 0 and c == 0:
                        emit_exp(pair, c, pre_sims[0])
                    else:
                        emit_exp(pair, c, emit_qk(pair, c))
                    if pair == 1 and c == 3:
                        # batch 1's AV block is deferred to here so that
                        # pair-1's first post-hoist QKs aren't queued
                        # behind 16 AV matmuls at the pair boundary
                        emit_avs(1, rv_pairs[0])
                if pair == 0:
                    # hoist pair-1's first two chunks ahead of pair-0's AV
                    # block so the exp stream doesn't stall at the boundary
                    emit_exp(1, 0, emit_qk(1, 0))
                    emit_exp(1, 1, emit_qk(1, 1))
                    done.add((1, 0))
                    done.add((1, 1))
                    emit_avs(0, rv_pairs[0])
                else:
                    emit_avs(2, rv_pairs[1])
                    emit_avs(3, rv_pairs[1])

            # hoist the gelu table load: the fake dependency on the last
            # exp tile pins it right after the exp stream, so it overlaps
            # the BN2 stats chain instead of sitting before the gelus
            nc.scalar.activation(warm_sb[:], exp_tiles[N - 1][NCH - 1][0:1, 0:1],
                                 AF.Gelu)

            # ---- BN2 + gelu epilogue (affine folded into Gelu) ----
            mv = sm.tile([64, 2], f32, tag="mv", bufs=1)
            nc.vector.bn_aggr(mv[:], stats[:])
            rsv = sm.tile([64, 1], f32, tag="rsv", bufs=1)
            dve_rsqrt(rsv[:], mv[:, 1:2], 64, magic64, 1, "v")
            scale_c = sm.tile([64, 1], f32, tag="sclc", bufs=1)
            nc.vector.tensor_tensor(out=scale_c[:], in0=rsv[:], in1=wv_sb[:],
                                    op=ALU.mult)
            mt = sm.tile([64, 1], f32, tag="mt", bufs=1)
            nc.vector.tensor_tensor(out=mt[:], in0=mv[:, 0:1], in1=scale_c[:],
                                    op=ALU.mult)
            bias_c = sm.tile([64, 1], f32, tag="bsc", bufs=1)
            nc.vector.tensor_tensor(out=bias_c[:], in0=bv_sb[:], in1=mt[:],
                                    op=ALU.subtract)

            # fp16 output halves the DMA bytes (~5e-4 rounding noise vs a
            # 2e-2 gate; the host upcasts to fp32)
            osb = outp.tile([64, N, L], f16, tag="osb", bufs=1)
            for b in range(N):
                nc.scalar.activation(osb[:, b], rv_tiles[b // 2][:, b % 2],
                                     AF.Gelu, bias=bias_c[:, 0:1],
                                     scale=scale_c[:, 0:1])
                # per-batch DMAs on the idle sync ring pipeline under the
                # remaining gelus
                nc.sync.dma_start(out_d.ap()[:, b], osb[:, b])

    nc.compile()
    return nc


def _host_inputs(query, key, value, bn_sim_weight, bn_sim_bias,
                 bn_val_weight, bn_val_bias, h):
    """Build the per-core (per-head) input map, with host-side layout prep."""
    f32 = np.float32
    f16 = np.float16
    qh = np.asarray(query[:, h], dtype=f32)   # [4, 64, 1024]
    kh = np.asarray(key[:, h], dtype=f32)
    vh = np.asarray(value[:, h], dtype=f32)

    def pack_pairs(x):
        # [4, 64, L] -> [128, 2, L]; row b_in*64+d, col (pair, l)
        return np.ascontiguousarray(
            x.reshape(2, 2, 64, L).transpose(1, 2, 0, 3).reshape(128, 2, L)
            .astype(f16))

    def chunked_t(x):
        # [4, 64, L] -> [128(m), 4(b), 8(chunk), 64]
        return x.transpose(2, 0, 1).reshape(NCH, 128, N, 64).transpose(
            1, 2, 0, 3)

    kq = np.empty((128, N, NG, 129), dtype=f16)
    kq[..., 0:64] = chunked_t(kh)[:, :, :NG].astype(f16)
    kq[..., 64:128] = chunked_t(qh)[:, :, :NG].astype(f16)
    kq[..., 128] = 1.0

    vo = np.empty((128, N, NCH, 65), dtype=f16)
    vo[..., :64] = chunked_t(vh).astype(f16)
    vo[..., 64] = 1.0

    ident = np.zeros((128, 64), dtype=f16)
    ident[64:128] = np.eye(64, dtype=f16)

    return {
        "ident": ident,
        "q2": pack_pairs(qh),
        "k2": pack_pairs(kh),
        "kqo": np.ascontiguousarray(kq),
        "vo": np.ascontiguousarray(vo),
        "ws": np.asarray(bn_sim_weight[h], dtype=f32).reshape(1, 1),
        "wv": np.ascontiguousarray(
            np.asarray(bn_val_weight[h * 64:(h + 1) * 64], dtype=f32)
            .reshape(64, 1)),
        "bv": np.ascontiguousarray(
            np.asarray(bn_val_bias[h * 64:(h + 1) * 64], dtype=f32)
            .reshape(64, 1)),
    }


def get_nc():
    if "nc" not in _CACHE:
        _CACHE["nc"] = _build_nc()
    return _CACHE["nc"]


def make_in_maps(**inputs):
    return [_host_inputs(
        inputs["query"], inputs["key"], inputs["value"],
        inputs["bn_sim_weight"], inputs["bn_sim_bias"],
        inputs["bn_val_weight"], inputs["bn_val_bias"], h) for h in range(H)]


def kernel(**inputs):
    from concourse.bass_utils import run_bass_kernel_spmd

    nc = get_nc()
    in_maps = make_in_maps(**inputs)
    res = run_bass_kernel_spmd(nc, in_maps, core_ids=list(range(H)))
    outs = [np.asarray(res.results[i]["out"]).transpose(1, 0, 2)
            for i in range(H)]
    return np.ascontiguousarray(
        np.concatenate(outs, axis=1).astype(np.float32))


# revision 44
# speedup vs baseline: 1.0722x; 1.0722x over previous
"""Trainium2 Bass kernel for nn_AttentionOperation (sparse_attention).

Computation (per the reference):
    sim  = QK^T                  [N,H,L,L]
    sim  = BN_heads(sim)         (stats over b,l,m per head)
    attn = softmax(sim, -1)
    rv   = attn @ V^T            [N,H,C,L] -> [N, H*C, L]
    rv   = BN_channels(rv)       (stats over b,l per channel)
    out  = gelu_exact(rv)

Sharding: one head per NeuronCore (H=8, n_cores=8).  Both BatchNorms are
then fully core-local (sim-BN stats are per head; val-BN channels
h*64..(h+1)*64-1 belong exactly to head h), so there is no communication.

Device-side structure (PE runs mostly at the throttled 1.2 GHz clock, so
the pair-stacked QK layout matters: batch b_in=0 lives on partitions
0:64 and b_in=1 on 64:128, which the PE runs as concurrent row-tiles):
  * BN1 mean/bias shift cancels inside the softmax, so only
    g = w_h * rsqrt(var + eps) is needed.  var comes from tiny Gram
    matmuls; kqo arrives in two per-batch-pair DMAs so the grams overlap
    the transfer of the second half.
  * matmul operands are fp16 (fp32 matmul = 2 half-rate passes on PE).
  * softmax denominator comes free from a ones-row appended to V^T.
  * rsqrt is a DVE-only quake-seed Newton iteration (2 its for BN1, 1
    for BN2 - seed err 3.4%, 1 it -> 1.7e-3), so ScalarE needs only the
    Exp and Gelu table sets (2 loads total).
  * BN2 affine is folded into the Gelu activation's scale/bias operands.
  * output is fp16 (host upcasts) and leaves via per-batch sync-ring
    DMAs that pipeline under the remaining gelus.
"""

import numpy as np

N, H, D, L = 4, 8, 64, 1024
C = 64
NCH = L // 128  # m-chunks of 128
EPS = 1e-3
NG = 8   # m-chunks in the BN1 gram stats (8 = exact reference stats)
CNT = float(N * (128 * NG) * (128 * NG))  # sampled sim pairs

_CACHE = {}


def _build_nc():
    import concourse.bacc as bacc
    import concourse.tile as tile
    import concourse.mybir as mybir

    f32 = mybir.dt.float32
    f16 = mybir.dt.float16
    f8 = mybir.dt.float8e4
    i32 = mybir.dt.int32
    AF = mybir.ActivationFunctionType
    ALU = mybir.AluOpType

    nc = bacc.Bacc("TRN2", target_bir_lowering=False, debug=False)

    q2_d = nc.dram_tensor("q2", [128, 2, L], f16, kind="ExternalInput")
    k2_d = nc.dram_tensor("k2", [128, 2, L], f16, kind="ExternalInput")
    kqo_d = nc.dram_tensor("kqo", [128, N, NG, 129], f8,
                           kind="ExternalInput")
    vo_d = nc.dram_tensor("vo", [128, N, NCH, 65], f16, kind="ExternalInput")
    id_d = nc.dram_tensor("ident", [128, 64], f16, kind="ExternalInput")
    ws_d = nc.dram_tensor("ws", [1, 1], f32, kind="ExternalInput")
    wv_d = nc.dram_tensor("wv", [64, 1], f32, kind="ExternalInput")
    bv_d = nc.dram_tensor("bv", [64, 1], f32, kind="ExternalInput")
    out_d = nc.dram_tensor("out", [64, N, L], f16, kind="ExternalOutput")

    with tile.TileContext(nc) as tc:
        with (
            tc.tile_pool(name="cst", bufs=1) as cst,
            tc.tile_pool(name="sm", bufs=1) as sm,
            tc.tile_pool(name="exp", bufs=24) as epool,
            tc.tile_pool(name="rvp", bufs=4) as rvp,
            tc.tile_pool(name="outp", bufs=4) as outp,
            tc.tile_pool(name="ps", bufs=1, space="PSUM") as psp,
        ):
            ones64 = cst.tile([64, 1], f32)
            nc.vector.memset(ones64[:], 1.0)
            magic1 = cst.tile([1, 1], i32)
            nc.vector.memset(magic1[:], 0x5F3759DF)
            magic64 = cst.tile([64, 1], i32)
            nc.vector.memset(magic64[:], 0x5F3759DF)
            # dummy exp so the ACT exp-table load happens off the critical
            # path (otherwise it lands right before the first real exp)
            warm_sb = sm.tile([1, 1], f32, tag="warm", bufs=1)
            nc.scalar.activation(warm_sb[:], ones64[0:1, 0:1], AF.Exp)

            # ---- input DMAs.  The sync ring drains FIFO, so issue in
            # dependency order: kqo halves (gate the grams), then pair-0
            # q/k (gates the hoisted first QK chunks), then the tiny
            # tensors the g-chain needs, then the rest.
            kqo_sb = cst.tile([128, N, NG, 129], f8)
            nc.sync.dma_start(kqo_sb[:, 0:2], kqo_d.ap()[:, 0:2])
            # second half on the scalar HWDGE ring: the halves transfer
            # concurrently, so grams for batches 2/3 start ~1.5us earlier
            nc.scalar.dma_start(kqo_sb[:, 2:4], kqo_d.ap()[:, 2:4])
            id_sb = cst.tile([128, 64], f16)
            nc.sync.dma_start(id_sb[:], id_d.ap())
            ws_sb = cst.tile([1, 1], f32)
            nc.sync.dma_start(ws_sb[:], ws_d.ap())
            q2_sb = cst.tile([128, 2, L], f16)
            k2_sb = cst.tile([128, 2, L], f16)
            nc.sync.dma_start(k2_sb[:, 0], k2_d.ap()[:, 0])
            nc.sync.dma_start(q2_sb[:, 0], q2_d.ap()[:, 0])
            nc.sync.dma_start(k2_sb[:, 1], k2_d.ap()[:, 1])
            nc.sync.dma_start(q2_sb[:, 1], q2_d.ap()[:, 1])
            wv_sb = cst.tile([64, 1], f32)
            nc.sync.dma_start(wv_sb[:], wv_d.ap())
            bv_sb = cst.tile([64, 1], f32)
            nc.sync.dma_start(bv_sb[:], bv_d.ap())
            vo_sb = cst.tile([128, N, NCH, 65], f16)
            nc.sync.dma_start(vo_sb[:, 0:2], vo_d.ap()[:, 0:2])
            nc.sync.dma_start(vo_sb[:, 2:4], vo_d.ap()[:, 2:4])

            # ---- BN1 stats: one stacked gram matmul per (batch, chunk).
            # G[b] = [k|q|1]^T [k|q|1]:  C_K = G[0:64,0:64],
            # C_Q = G[64:128,64:128], ksum = G[0:64,128], qsum = G[64:128,128]
            # Everything is done per batch so only batch 3's realign +
            # product chain sits on the critical path after its gram.
            qsp = sm.tile([64, 2], f32, tag="qs", bufs=1)
            sprod = sm.tile([64, N], f32, tag="sprod", bufs=1)
            kk_sb = sm.tile([64, N, 129], f32, tag="kk", bufs=1)
            qsrc_sb = sm.tile([128, N, 65], f16, tag="gk", bufs=1)
            for b in range(N):
                gps = psp.tile([128, 129], f32, tag="av", bufs=2,
                               name=f"gram_ps_{b}")
                for c in range(NG):
                    nc.tensor.matmul(
                        gps[:], kqo_sb[:, b, c, 0:128], kqo_sb[:, b, c, :],
                        start=(c == 0), stop=(c == NG - 1))
                nc.scalar.copy(kk_sb[:, b, :], gps[0:64, :])
                nc.vector.tensor_copy(qsrc_sb[64:128, b, :],
                                      gps[64:128, 64:129])
            qq_ps = psp.tile([64, N, 65], f32, tag="av", bufs=2, name="qq")
            nc.tensor.matmul(qq_ps[:], id_sb[64:128, :],
                             qsrc_sb[64:128, :, :], start=True, stop=True)
            # products read C_Q straight from PSUM (DVE has a PSUM port)
            pscr = sm.tile([64, N, 64], f32, tag="pscr", bufs=1)
            nc.vector.tensor_tensor(
                out=pscr[:], in0=kk_sb[:, :, 0:64], in1=qq_ps[:, :, 0:64],
                op=ALU.mult)
            nc.vector.tensor_reduce(
                out=qsp[:, 0:1], in_=pscr[:],
                axis=mybir.AxisListType.XY, op=ALU.add)
            nc.vector.tensor_tensor(
                out=sprod[:], in0=kk_sb[:, :, 128], in1=qq_ps[:, :, 64],
                op=ALU.mult)
            nc.vector.tensor_reduce(
                out=qsp[:, 1:2], in_=sprod[:],
                axis=mybir.AxisListType.X, op=ALU.add)

            # hoisted first QK chunks: keeps PE busy while the g-chain
            # (DVE scalar ops) runs, and has sim ready for the first exps
            def emit_qk(pair, c):
                sims = []
                for b_in in range(2):
                    b = 2 * pair + b_in
                    r0 = 64 * b_in
                    sim_ps = psp.tile([128, L], f32, tag="sim", bufs=3,
                                      name=f"sim_ps_{b}_{c}")
                    for half in range(2):
                        nc.tensor.matmul(
                            sim_ps[:, 512 * half:512 * (half + 1)],
                            k2_sb[r0:r0 + 64, pair, 128 * c:128 * (c + 1)],
                            q2_sb[r0:r0 + 64, pair,
                                  512 * half:512 * (half + 1)],
                            start=True, stop=True)
                    sims.append(sim_ps)
                return sims

            pre_sims = [emit_qk(0, 0)]

            # partition-sum via PE, then fold the 4 per-batch slots:
            # qs2 [1,2] = [sum(sim^2), sum(sim)]
            scps = psp.tile([1, 2], f32, tag="av", bufs=2)
            nc.tensor.matmul(scps[:], ones64[:], qsp[:], start=True,
                             stop=True)
            qs2 = sm.tile([1, 2], f32, tag="qs2", bufs=1)
            nc.vector.tensor_copy(qs2[:], scps[:])

            # DVE-only rsqrt(x + eps): quake seed + Newton iterations.
            def dve_rsqrt(dst_ap, x_ap, p, magic, n_it, pref,
                          add_eps=True):
                if add_eps:
                    xe_t = sm.tile([p, 1], f32, tag=f"{pref}xe", bufs=1,
                                   name=f"{pref}_xe")
                    nc.vector.tensor_scalar_add(xe_t[:], x_ap, EPS)
                    xe = xe_t[:]
                else:
                    xe = x_ap
                sh = sm.tile([p, 1], i32, tag=f"{pref}sh", bufs=1,
                             name=f"{pref}_sh")
                nc.vector.tensor_scalar(
                    out=sh[:], in0=xe.bitcast(i32), scalar1=1,
                    scalar2=None, op0=ALU.arith_shift_right)
                y = sm.tile([p, 1], f32, tag=f"{pref}y", bufs=1,
                            name=f"{pref}_y")
                nc.vector.tensor_tensor(out=y[:].bitcast(i32), in0=magic[:],
                                        in1=sh[:], op=ALU.subtract)
                t = sm.tile([p, 1], f32, tag=f"{pref}t", bufs=1,
                            name=f"{pref}_t")
                for it in range(n_it):
                    nc.vector.tensor_tensor(out=t[:], in0=y[:], in1=y[:],
                                            op=ALU.mult)
                    nc.vector.scalar_tensor_tensor(
                        out=t[:], in0=t[:], scalar=-0.5, in1=xe,
                        op0=ALU.mult, op1=ALU.mult)
                    nc.vector.scalar_tensor_tensor(
                        out=(dst_ap if it == n_it - 1 else y[:]), in0=t[:],
                        scalar=1.5, in1=y[:], op0=ALU.add, op1=ALU.mult)

            # var = E[x^2] - E[x]^2 ; g = w_h * rsqrt(var + eps)
            m2_t = sm.tile([1, 1], f32, tag="sc1", bufs=1)
            nc.vector.scalar_tensor_tensor(
                out=m2_t[:], in0=qs2[:, 1:2], scalar=1.0 / (CNT * CNT),
                in1=qs2[:, 1:2], op0=ALU.mult, op1=ALU.mult)
            var_t = sm.tile([1, 1], f32, tag="sc2", bufs=1)
            nc.vector.scalar_tensor_tensor(
                out=var_t[:], in0=qs2[:, 0:1], scalar=1.0 / CNT,
                in1=m2_t[:], op0=ALU.mult, op1=ALU.subtract)
            rs_t = sm.tile([1, 1], f32, tag="sc3", bufs=1)
            dve_rsqrt(rs_t[:], var_t[:], 1, magic1, 2, "g",
                      add_eps=False)
            g_t = sm.tile([1, 1], f32, tag="sc4", bufs=1)
            nc.vector.tensor_tensor(out=g_t[:], in0=rs_t[:], in1=ws_sb[:],
                                    op=ALU.mult)
            # broadcast g to all 128 partitions (gpsimd, ~300ns)
            g128 = cst.tile([128, 1], f32)
            nc.gpsimd.partition_broadcast(g128[:], g_t[:], channels=128)

            # ---- main attention pipeline ----
            exp_tiles = [[None] * NCH for _ in range(N)]
            rv_tiles = []
            stats = cst.tile([64, 2 * N, 6], f32)

            def emit_exp(pair, c, sims):
                for b_in in range(2):
                    b = 2 * pair + b_in
                    ex = epool.tile([128, L], f16, tag="exp", bufs=24,
                                    name=f"exp_{b}_{c}")
                    nc.scalar.activation(ex[:], sims[b_in][:], AF.Exp,
                                         scale=g128[:, 0:1])
                    exp_tiles[b][c] = ex

            def emit_avs(b, rv_pair):
                    b_in = b % 2
                    rcp_sb = sm.tile([1, L], f32, tag="rcp", bufs=4,
                                     name=f"rcp_{b}")
                    rbc_sb = sm.tile([64, L], f32, tag="rbc", bufs=4,
                                     name=f"rbc_{b}")
                    rv_sb = rv_pair[:, b_in]
                    for half in range(2):
                        hs = slice(512 * half, 512 * (half + 1))
                        av_ps = psp.tile([65, 512], f32, tag="av", bufs=2,
                                         name=f"av_ps_{b}_{half}")
                        for c in range(NCH):
                            nc.tensor.matmul(
                                av_ps[:], vo_sb[:, b, c, :],
                                exp_tiles[b][c][:, hs],
                                start=(c == 0), stop=(c == NCH - 1))
                        # den row to partition 0 (custom-DVE recip needs a
                        # base-0 SBUF operand), then a full-tile copy --
                        # same DVE cost as a 1-row copy (free-dim-bound) --
                        # to release the PSUM slot for the next batch's AV
                        den_sb = sm.tile([1, 512], f32, tag="den", bufs=4,
                                         name=f"den_{b}_{half}")
                        nc.vector.tensor_copy(den_sb[:], av_ps[64:65, :])
                        if b < N - 1:
                            # early PSUM release matters only when another
                            # batch's AV still needs the slot
                            av_sb = sm.tile([65, 512], f32, tag="avs",
                                            bufs=6, name=f"av_sb_{b}_{half}")
                            nc.vector.tensor_copy(av_sb[:], av_ps[:])
                            num = av_sb[0:64, :]
                        else:
                            num = av_ps[0:64, :]
                        nc.vector.reciprocal_approx_fast(
                            out=rcp_sb[0:1, hs], in_=den_sb[:])
                        nc.gpsimd.partition_broadcast(
                            rbc_sb[:, hs], rcp_sb[0:1, hs], channels=64)
                        nc.vector.tensor_tensor(
                            out=rv_sb[:, hs], in0=num,
                            in1=rbc_sb[:, hs], op=ALU.mult)
                        nc.vector.bn_stats(stats[:, 2 * b + half, :],
                                           rv_sb[:, hs])

            done = set()
            rv_pairs = [rvp.tile([64, 2, L], f32, tag="rv", bufs=2,
                                 name=f"rv_pair_{p}") for p in range(2)]
            for pair in range(2):
                rv_tiles.append(rv_pairs[pair])
                for c in range(NCH):
                    if (pair, c) in done:
                        continue
                    if pair == 0 and c == 0:
                        emit_exp(pair, c, pre_sims[0])
                    else:
                        emit_exp(pair, c, emit_qk(pair, c))
                if pair == 0:
                    # hoist pair-1's first two chunks ahead of pair-0's AV
                    # block so the exp stream doesn't stall at the boundary
                    emit_exp(1, 0, emit_qk(1, 0))
                    emit_exp(1, 1, emit_qk(1, 1))
                    done.add((1, 0))
                    done.add((1, 1))
                    emit_avs(0, rv_pairs[0])
                    emit_avs(1, rv_pairs[0])
                else:
                    emit_avs(2, rv_pairs[1])
                    emit_avs(3, rv_pairs[1])

            # hoist the gelu table load: the fake dependency on the last
            # exp tile pins it right after the exp stream, so it overlaps
            # the BN2 stats chain instead of sitting before the gelus
            nc.scalar.activation(warm_sb[:], exp_tiles[N - 1][NCH - 1][0:1, 0:1],
                                 AF.Gelu)

            # ---- BN2 + gelu epilogue (affine folded into Gelu) ----
            mv = sm.tile([64, 2], f32, tag="mv", bufs=1)
            nc.vector.bn_aggr(mv[:], stats[:])
            rsv = sm.tile([64, 1], f32, tag="rsv", bufs=1)
            dve_rsqrt(rsv[:], mv[:, 1:2], 64, magic64, 1, "v")
            scale_c = sm.tile([64, 1], f32, tag="sclc", bufs=1)
            nc.vector.tensor_tensor(out=scale_c[:], in0=rsv[:], in1=wv_sb[:],
                                    op=ALU.mult)
            mt = sm.tile([64, 1], f32, tag="mt", bufs=1)
            nc.vector.tensor_tensor(out=mt[:], in0=mv[:, 0:1], in1=scale_c[:],
                                    op=ALU.mult)
            bias_c = sm.tile([64, 1], f32, tag="bsc", bufs=1)
            nc.vector.tensor_tensor(out=bias_c[:], in0=bv_sb[:], in1=mt[:],
                                    op=ALU.subtract)

            # fp16 output halves the DMA bytes (~5e-4 rounding noise vs a
            # 2e-2 gate; the host upcasts to fp32)
            osb = outp.tile([64, N, L], f16, tag="osb", bufs=1)
            for b in range(N):
                nc.scalar.activation(osb[:, b], rv_tiles[b // 2][:, b % 2],
                                     AF.Gelu, bias=bias_c[:, 0:1],
                                     scale=scale_c[:, 0:1])
                # per-batch DMAs on the idle sync ring pipeline under the
                # remaining gelus
                nc.sync.dma_start(out_d.ap()[:, b], osb[:, b])

    nc.compile()
    return nc


def _host_inputs(query, key, value, bn_sim_weight, bn_sim_bias,
                 bn_val_weight, bn_val_bias, h):
    """Build the per-core (per-head) input map, with host-side layout prep."""
    f32 = np.float32
    f16 = np.float16
    qh = np.asarray(query[:, h], dtype=f32)   # [4, 64, 1024]
    kh = np.asarray(key[:, h], dtype=f32)
    vh = np.asarray(value[:, h], dtype=f32)

    def pack_pairs(x):
        # [4, 64, L] -> [128, 2, L]; row b_in*64+d, col (pair, l)
        return np.ascontiguousarray(
            x.reshape(2, 2, 64, L).transpose(1, 2, 0, 3).reshape(128, 2, L)
            .astype(f16))

    def chunked_t(x):
        # [4, 64, L] -> [128(m), 4(b), 8(chunk), 64]
        return x.transpose(2, 0, 1).reshape(NCH, 128, N, 64).transpose(
            1, 2, 0, 3)

    import ml_dtypes
    f8 = ml_dtypes.float8_e4m3
    kq = np.empty((128, N, NG, 129), dtype=f8)
    kq[..., 0:64] = chunked_t(kh)[:, :, :NG].astype(f8)
    kq[..., 64:128] = chunked_t(qh)[:, :, :NG].astype(f8)
    kq[..., 128] = 1.0

    vo = np.empty((128, N, NCH, 65), dtype=f16)
    vo[..., :64] = chunked_t(vh).astype(f16)
    vo[..., 64] = 1.0

    ident = np.zeros((128, 64), dtype=f16)
    ident[64:128] = np.eye(64, dtype=f16)

    return {
        "ident": ident,
        "q2": pack_pairs(qh),
        "k2": pack_pairs(kh),
        "kqo": np.ascontiguousarray(kq),
        "vo": np.ascontiguousarray(vo),
        "ws": np.asarray(bn_sim_weight[h], dtype=f32).reshape(1, 1),
        "wv": np.ascontiguousarray(
            np.asarray(bn_val_weight[h * 64:(h + 1) * 64], dtype=f32)
            .reshape(64, 1)),
        "bv": np.ascontiguousarray(
            np.asarray(bn_val_bias[h * 64:(h + 1) * 64], dtype=f32)
            .reshape(64, 1)),
    }


def get_nc():
    if "nc" not in _CACHE:
        _CACHE["nc"] = _build_nc()
    return _CACHE["nc"]


def make_in_maps(**inputs):
    return [_host_inputs(
        inputs["query"], inputs["key"], inputs["value"],
        inputs["bn_sim_weight"], inputs["bn_sim_bias"],
        inputs["bn_val_weight"], inputs["bn_val_bias"], h) for h in range(H)]


def kernel(**inputs):
    from concourse.bass_utils import run_bass_kernel_spmd

    nc = get_nc()
    in_maps = make_in_maps(**inputs)
    res = run_bass_kernel_spmd(nc, in_maps, core_ids=list(range(H)))
    outs = [np.asarray(res.results[i]["out"]).transpose(1, 0, 2)
            for i in range(H)]
    return np.ascontiguousarray(
        np.concatenate(outs, axis=1).astype(np.float32))
